# revision 27
# baseline (speedup 1.0000x reference)
"""Trainium2 Bass kernel: Mixture-of-Experts SwiGLU feed-forward.

Module: x:[4,2048,512] -> router top-2-of-8 (softmax over selected
logits) -> per-expert SwiGLU FFN (h=silu(x@W1)*(x@W3); y=h@W2) ->
weighted combine.

Sharding (expert-parallel, per the hint): the host computes the router
(cheap: 8192x512x8 matmul + top-2), dispatches each expert's tokens to
the core owning that expert (all-to-all dispatch by top-k expert id),
each of the 8 NeuronCores runs its expert's FFN over a fixed-capacity
token batch (capacity factor 1.0 = 2048 tokens), and the host applies
gate weights and scatter-adds the expert outputs back into the full
output (weighted all-to-all return). The few tokens past an expert's
capacity (load imbalance remainder, ~1% of traffic) are computed on
the host instead of being dropped.

Per-core compute is 384 bf16 [128x128]x[128x512] PE matmuls = 82us of
pure streaming at 2.4GHz, and the schedule is built to keep the PE at
that floor end to end:

- The PE is kept continuously busy from engine-init (~7us) through the
  HAM clock-ramp window (~3.4us of sustained activity before the PE
  un-throttles 1.2->2.4GHz): dummy warmup matmuls fill the span before
  the first DMA'd operands land and are interleaved into the early
  supply gaps so the activity window never resets. Without this the
  first ~15 real matmuls run at half clock.
- DMA is issued on all three DGE queues (sync/scalar HWDGE, gpsimd
  SWDGE) in consumption order: the first x chunk and W1[m0] go first
  and land ~8.2us, weight tiles stay >=2 m-chunks ahead of the PE, x
  and W2 for later blocks stage during block-0 compute, outputs flush
  per 512-token block on the gpsimd queue during compute.
- The L1->L2 transition inside each token block stalls the PE on the
  last hidden chunk's silu*mul (vector op) before the first W2 matmul
  group can accumulate; the first W1 group of the NEXT block is
  emitted in between to cover that latency (and symmetrically removes
  the L2->L1 bubble). The last block has no successor, so its output
  matmul groups are split into 256-token halves: the first half's
  store chain starts ~1us before the final matmul retires, shrinking
  the post-compute flush tail (DMA-to-HBM completion receipt ~1us).

On-device compute uses bf16 matmuls (full-rate on the TRN2 PE, ~5e-3
relative error vs the 2e-2 gate) with fp32 PSUM accumulation; fp8
DoubleRow (1.44x) was measured numerically to land at 4.5-6.8e-2 error
in every variant, over the gate, so bf16 is the fastest legal dtype.
Activations live transposed ([feature, token]) on device so every
matmul consumes naturally-laid-out weights as the stationary operand
and no on-device transposes are needed. Weights are host-permuted
m-major so each DMA fetches exactly the 128-column block the next
psum group needs; x/y are host-permuted (k,block)/(block,j)-major so
every transfer is a contiguous 128KB DRAM range.
"""

import os
import sys
import types

for _p in ("/opt/trn_rl_repo",):
    if os.path.isdir(_p) and _p not in sys.path:
        sys.path.insert(0, _p)

import numpy as np
import ml_dtypes

BF16 = ml_dtypes.bfloat16

# Problem dims (fixed by the nn.Module spec)
D = 512          # d_model
H = 1024         # ffn hidden
E = 8            # experts
TOPK = 2
T = 8192         # tokens = 4*2048
P = 128          # SBUF partitions
CAP = 2048       # per-expert token capacity (capacity factor 1.0)
BT = 512         # token block (moving operand / PSUM bank limit)
NB = CAP // BT   # 4 token blocks of 512
DK = D // P      # 4 contraction chunks over d
MH = H // P      # 8 hidden chunks
N_CORES = 8

_compiled = {}
last_exec_time_ns = None
last_results = None


def _install_axon_trace_shim():
    """Make trace=True under axon survive images without antenv.axon_hooks."""
    try:
        import antenv  # noqa: F401
    except Exception:
        return
    try:
        from antenv import axon_hooks  # noqa: F401
        return  # real module present
    except Exception:
        pass
    try:
        import antenv
        boot_dir = "/root/.axon_site/trn_agent_boot"
        if os.path.isdir(boot_dir) and boot_dir not in sys.path:
            sys.path.insert(0, boot_dir)
        import trn_boot
        mod = types.ModuleType("antenv.axon_hooks")
        holder = {"hook": trn_boot._ntff_profile_via_ctypes("/opt/axon/libaxon_pjrt.so")}
        mod.set_axon_ntff_profile_hook = lambda h: holder.__setitem__("hook", h)
        mod.get_axon_ntff_profile_hook = lambda: holder["hook"]
        sys.modules["antenv.axon_hooks"] = mod
        antenv.axon_hooks = mod
    except Exception:
        pass


def _patch_upload_artifacts():
    """Artifact upload needs fishnet; degrade to the local dir if absent."""
    try:
        import concourse.bass_utils as bu
        orig = bu.upload_artifacts

        def safe_upload(tmpdir):
            try:
                return orig(tmpdir)
            except Exception:
                return tmpdir

        if getattr(bu.upload_artifacts, "__name__", "") != "safe_upload":
            bu.upload_artifacts = safe_upload
    except Exception:
        pass


def _build():
    from concourse import bacc, mybir
    import concourse.tile as tile

    f32 = mybir.dt.float32
    bf16 = mybir.dt.bfloat16

    nc = bacc.Bacc(num_swdge_queues=1)
    # x chunk-major: (k, b) block is a contiguous [P, BT] 128KB DRAM range
    xd = nc.declare_dram_parameter("xd", [DK * NB * P, BT], bf16, isOutput=False)
    w1 = nc.declare_dram_parameter("w1", [MH * P, DK * P], bf16, isOutput=False)
    w3 = nc.declare_dram_parameter("w3", [MH * P, DK * P], bf16, isOutput=False)
    w2 = nc.declare_dram_parameter("w2", [MH * P, D], bf16, isOutput=False)
    # y block-major: (b, j) block contiguous
    yd = nc.declare_dram_parameter("yd", [NB * DK * P, BT], bf16, isOutput=True)

    with tile.TileContext(nc) as tc:
        with tc.tile_pool(name="wpool", bufs=1) as wpool, \
             tc.tile_pool(name="act", bufs=2) as act, \
             tc.tile_pool(name="psum", bufs=1, space="PSUM") as psum:

            w1s = wpool.tile([P, MH, DK * P], bf16, tag="w1s")
            w3s = wpool.tile([P, MH, DK * P], bf16, tag="w3s")
            w2s = wpool.tile([P, MH, D], bf16, tag="w2s")
            # One tile per k-chunk: tile-granular dependency tracking
            # means a matmul depends on EVERY write to its tile emitted
            # before it, so finer tiles -> fewer false waits.
            xsk = [wpool.tile([P, CAP], bf16, tag=f"xs{k}", name=f"xs{k}")
                   for k in range(DK)]

            xv = xd[:].rearrange("(k b p) t -> k b p t", b=NB, p=P)
            w1v = w1[:].rearrange("(m p) c -> m p c", p=P)
            w3v = w3[:].rearrange("(m p) c -> m p c", p=P)
            w2v = w2[:].rearrange("(m p) d -> m p d", p=P)
            yv = yd[:].rearrange("(b j p) t -> b j p t", j=DK, p=P)

            def stage_x(eng, k, b):
                return eng.dma_start(out=xsk[k][:, BT * b:BT * (b + 1)],
                                     in_=xv[k, b])

            sil_insts = []

            # PE warmup weights; memset on gpsimd (ready earliest, and
            # keeps vector/scalar free for their first real ops).
            wscr = wpool.tile([P, P], bf16, tag="wscr")
            nc.gpsimd.memset(wscr[:], 0)
            warm_ps = psum.tile([P, P], f32, tag="warm")

            def warm(n):
                for _ in range(n):
                    nc.tensor.matmul(out=warm_ps[:], lhsT=wscr[:], rhs=wscr[:],
                                     start=True, stop=True)

            # First DMA wave, in PE consumption order. The critical
            # block-0 x path rides the two HWDGE queues (faster
            # completion receipt than SWDGE); the scalar queue gets ONLY
            # two early transfers so the silu chain never sits behind a
            # pile of ~600ns dma_start issues (the PE can run at most 2
            # hidden-chunks ahead of silu/mul on the ps1/ps2 slots).
            # Every other staging transfer is emitted as LATE as
            # possible: a consumer picks up a dependency on every write
            # to its tile emitted before it, while the issuing engine
            # still starts the transfer as early as its own FIFO allows.
            stage_x(nc.sync, 0, 0)
            nc.scalar.dma_start(out=w1s[:, 0], in_=w1v[0])
            nc.gpsimd.dma_start(out=w3s[:, 0], in_=w3v[0])
            # Dummy matmuls keep the PE busy (HAM activity window) while
            # the first operands are in flight (~8.2us arrival).
            warm(16)
            stage_x(nc.sync, 1, 0)
            stage_x(nc.scalar, 2, 0)
            stage_x(nc.sync, 3, 0)

            ht = wpool.tile([P, MH, BT], bf16, tag="ht", bufs=2)

            def l1_group(tag, ws, m, b, fills=None):
                ps = psum.tile([P, BT], f32, tag=tag, bufs=2)
                tok = slice(BT * b, BT * (b + 1))
                for k in range(DK):
                    nc.tensor.matmul(out=ps[:], lhsT=ws[:, m, k * P:(k + 1) * P],
                                     rhs=xsk[k][:, tok],
                                     start=(k == 0), stop=(k == DK - 1))
                    if fills:
                        warm(fills[k])
                return ps

            pending_ps1 = None
            for b in range(NB):
                for m in range(MH):
                    if pending_ps1 is not None:
                        ps1, pending_ps1 = pending_ps1, None
                    elif b == 0 and m == 0:
                        # warmup matmuls interleave into the arrival
                        # gaps of the bandwidth-paced first wave
                        ps1 = l1_group("ps1", w1s, m, b, fills=(3, 3, 2, 2))
                    else:
                        ps1 = l1_group("ps1", w1s, m, b)
                    if b == 0:
                        # Weight ladder, one chunk ahead of consumption:
                        # emitted right AFTER the last group that must
                        # not depend on it (tile-granular deps).
                        if m + 1 < MH:
                            nc.sync.dma_start(out=w1s[:, m + 1], in_=w1v[m + 1])
                        if m == 2:
                            stage_x(nc.sync, 0, 1)
                        if 3 <= m <= 6:
                            nc.sync.dma_start(out=w2s[:, 2 * (m - 3)],
                                              in_=w2v[2 * (m - 3)])
                    sil = act.tile([P, BT], f32, tag="sil")
                    sil_i = nc.scalar.activation(sil[:], ps1[:],
                                                 mybir.ActivationFunctionType.Silu)
                    if b == 0:
                        sil_insts.append(sil_i)
                    ps2 = l1_group("ps2", w3s, m, b)

                    def paced(dma_i, dep_m):
                        # The gpsimd engine free-runs its DMA FIFO; an
                        # explicit dep on an earlier silu paces the
                        # non-critical staging so it doesn't steal HBM
                        # bandwidth from the critical x/w1 stream in the
                        # first ~4us (the head is bandwidth-bound).
                        tile.add_dep_helper(
                            dma_i.ins, sil_insts[dep_m].ins, sync=True,
                            reason="pace staging behind compute")

                    if b == 0:
                        if m + 1 < MH:
                            d = nc.gpsimd.dma_start(out=w3s[:, m + 1],
                                                    in_=w3v[m + 1])
                            if m >= 1:
                                paced(d, m - 1)
                        if 1 <= m <= 3:
                            paced(stage_x(nc.gpsimd, m, 1), m)
                        if 4 <= m <= 7:
                            d = nc.gpsimd.dma_start(out=w2s[:, 2 * (m - 4) + 1],
                                                    in_=w2v[2 * (m - 4) + 1])
                            paced(d, m - 1)
                    elif b + 1 < NB and 1 <= m <= 4:
                        # stage next block's x during this block's L1
                        stage_x(nc.gpsimd, m - 1, b + 1)
                    if b == NB - 1 and m == MH - 1:
                        # split the last mul so the first half-width L2
                        # group can start ~350ns earlier
                        for h in range(2):
                            hs = slice(256 * h, 256 * (h + 1))
                            nc.vector.tensor_mul(out=ht[:, m, hs], in0=sil[:, hs],
                                                 in1=ps2[:, hs])
                    else:
                        nc.vector.tensor_mul(out=ht[:, m], in0=sil[:], in1=ps2[:])

                if b + 1 < NB:
                    # Borrow the next block's first W1 group: covers the
                    # silu*mul latency of this block's last hidden chunk
                    # so the first psy matmul never stalls the PE.
                    pending_ps1 = l1_group("ps1", w1s, 0, b + 1)

                tok = slice(BT * b, BT * (b + 1))
                if b < NB - 1:
                    for j in range(DK):
                        psy = psum.tile([P, BT], f32, tag="psy", bufs=2)
                        for m in range(MH):
                            nc.tensor.matmul(out=psy[:], lhsT=w2s[:, m, j * P:(j + 1) * P],
                                             rhs=ht[:, m, :],
                                             start=(m == 0), stop=(m == MH - 1))
                        yt = act.tile([P, BT], bf16, tag="yt")
                        nc.vector.tensor_copy(out=yt[:], in_=psy[:])
                        nc.gpsimd.dma_start(out=yv[b, j], in_=yt[:])
                else:
                    # Last block: half-width (256-token) L2 groups so the
                    # store chain overlaps the final matmuls and the very
                    # last transfer is small + issued on an idle HW queue.
                    for j in range(DK):
                        for h in range(2):
                            hs = slice(256 * h, 256 * (h + 1))
                            psy = psum.tile([P, 256], f32, tag="psy", bufs=2)
                            for m in range(MH):
                                nc.tensor.matmul(out=psy[:],
                                                 lhsT=w2s[:, m, j * P:(j + 1) * P],
                                                 rhs=ht[:, m, hs],
                                                 start=(m == 0), stop=(m == MH - 1))
                            yt = act.tile([P, 256], bf16, tag="yth", bufs=4)
                            nc.vector.tensor_copy(out=yt[:], in_=psy[:])
                            # idle HWDGE queues: faster completion receipt
                            # than SWDGE, and 4 yth slots mean no cast
                            # ever waits on a store two groups back.
                            eng = nc.scalar if (2 * j + h) % 2 else nc.sync
                            if j == DK - 1:
                                eng = nc.scalar if h == 0 else nc.sync
                            eng.dma_start(out=yv[b, j][:, hs], in_=yt[:])

    nc.compile()
    return nc


def _route(x2d, Wg, bg):
    """Replicate the reference router on host.

    Selection runs in float64 (agrees with the reference's fp32 jax
    selection whenever top-2/top-3 logit gaps exceed fp32 matmul noise,
    which holds with >10x margin on this distribution); the softmax over
    the two selected logits runs in fp32 like the reference.
    """
    logits64 = x2d.astype(np.float64) @ Wg.astype(np.float64) + bg.astype(np.float64)
    i1 = np.argmax(logits64, axis=1)
    r = np.arange(T)
    masked = logits64.copy()
    masked[r, i1] = -np.inf
    i2 = np.argmax(masked, axis=1)

    # fp32 logit values for the softmax (match reference arithmetic)
    logits32 = (x2d @ Wg + bg).astype(np.float32)
    v1 = logits32[r, i1]
    v2 = logits32[r, i2]
    # softmax over [v1, v2] with v1 >= v2 (fp32)
    e2 = np.exp((v2 - v1).astype(np.float32))
    p1 = (1.0 / (1.0 + e2)).astype(np.float32)
    p2 = (e2 / (1.0 + e2)).astype(np.float32)
    return i1, i2, p1, p2


def _ffn_host(x2d, idx, W1e, W3e, W2e):
    """Exact fp32 SwiGLU FFN for a small set of tokens (overflow path)."""
    z = x2d[idx] @ W1e
    h = (z / (1.0 + np.exp(-z))) * (x2d[idx] @ W3e)
    return h @ W2e


def kernel(x, Wg, bg, W1, W3, W2):
    global last_exec_time_ns
    _install_axon_trace_shim()
    _patch_upload_artifacts()
    from concourse.bass_utils import run_bass_kernel_spmd

    x = np.asarray(x, np.float32)
    Wg = np.asarray(Wg, np.float32)
    bg = np.asarray(bg, np.float32)
    W1 = np.asarray(W1, np.float32)
    W3 = np.asarray(W3, np.float32)
    W2 = np.asarray(W2, np.float32)

    B, S, _ = x.shape
    x2d = np.ascontiguousarray(x.reshape(T, D))

    i1, i2, p1, p2 = _route(x2d, Wg, bg)

    # Dispatch: build each expert's token list + gate weights. Tokens past
    # CAP (load-imbalance remainder) fall to the exact host path.
    idx_lists, gate_lists = [], []
    spill_lists = []
    for e in range(E):
        m1 = i1 == e
        m2 = i2 == e
        idx = np.concatenate([np.nonzero(m1)[0], np.nonzero(m2)[0]])
        g = np.concatenate([p1[m1], p2[m2]]).astype(np.float32)
        if len(idx) > CAP:
            # Spill the smallest-gate tokens: they matter least if anything
            # about the two paths' rounding ever differs.
            order = np.argsort(-g, kind="stable")
            idx, g = idx[order], g[order]
            spill_lists.append((idx[CAP:], g[CAP:]))
            idx, g = idx[:CAP], g[:CAP]
        else:
            spill_lists.append((idx[:0], g[:0]))
        idx_lists.append(idx)
        gate_lists.append(g)

    x2dT_bf = np.ascontiguousarray(x2d.T.astype(BF16))  # [D, T]
    in_maps = []
    for e in range(E):
        idx = idx_lists[e]
        xe = np.zeros((D, CAP), BF16)
        xe[:, : len(idx)] = x2dT_bf[:, idx]
        # chunk-major x: [DK, NB, P, BT] so each (k, b) DMA is contiguous
        xkb = np.ascontiguousarray(
            xe.reshape(DK, P, NB, BT).transpose(0, 2, 1, 3)
        ).reshape(DK * NB * P, BT)
        # m-major weight layout: [MH, P, DK*128]
        w1m = np.ascontiguousarray(
            W1[e].astype(BF16).reshape(DK, P, MH, P).transpose(2, 1, 0, 3)
        ).reshape(MH * P, DK * P)
        w3m = np.ascontiguousarray(
            W3[e].astype(BF16).reshape(DK, P, MH, P).transpose(2, 1, 0, 3)
        ).reshape(MH * P, DK * P)
        in_maps.append({
            "xd": xkb,
            "w1": w1m,
            "w3": w3m,
            "w2": np.ascontiguousarray(W2[e].astype(BF16)),
        })

    if "nc" not in _compiled:
        _compiled["nc"] = _build()
    nc = _compiled["nc"]

    trace = bool(os.environ.get("BASS_TRACE"))
    res = run_bass_kernel_spmd(nc, in_maps, list(range(N_CORES)), trace=trace)
    last_exec_time_ns = res.exec_time_ns
    globals()["last_results"] = res

    y = np.zeros((T, D), np.float32)
    for e in range(E):
        idx = idx_lists[e]
        n = len(idx)
        # yd is [NB, DK, P, BT] block-major; back to [D, CAP]
        ye = np.asarray(res.results[e]["yd"]).reshape(NB, DK, P, BT)
        ye = ye.transpose(1, 2, 0, 3).reshape(D, CAP)
        y[idx] += gate_lists[e][:, None] * ye[:, :n].T.astype(np.float32)
        sidx, sg = spill_lists[e]
        if len(sidx):
            y[sidx] += sg[:, None] * _ffn_host(x2d, sidx, W1[e], W3[e], W2[e])
    return y.reshape(B, S, D)


# revision 28
# speedup vs baseline: 1.0410x; 1.0410x over previous
"""Trainium2 Bass kernel: Mixture-of-Experts SwiGLU feed-forward.

Module: x:[4,2048,512] -> router top-2-of-8 (softmax over selected
logits) -> per-expert SwiGLU FFN (h=silu(x@W1)*(x@W3); y=h@W2) ->
weighted combine.

Sharding (expert-parallel, per the hint): the host computes the router
(cheap: 8192x512x8 matmul + top-2), dispatches each expert's tokens to
the core owning that expert (all-to-all dispatch by top-k expert id),
each of the 8 NeuronCores runs its expert's FFN over a fixed-capacity
token batch (capacity factor 1.0 = 2048 tokens), and the host applies
gate weights and scatter-adds the expert outputs back into the full
output (weighted all-to-all return). The few tokens past an expert's
capacity (load imbalance remainder, ~1% of traffic) are computed on
the host instead of being dropped.

Per-core compute is 384 bf16 [128x128]x[128x512] PE matmuls = 82us of
pure streaming at 2.4GHz, and the schedule is built to keep the PE at
that floor end to end:

- The PE is kept continuously busy from engine-init (~7us) through the
  HAM clock-ramp window (~3.4us of sustained activity before the PE
  un-throttles 1.2->2.4GHz): dummy warmup matmuls fill the span before
  the first DMA'd operands land and are interleaved into the early
  supply gaps so the activity window never resets. Without this the
  first ~15 real matmuls run at half clock.
- DMA is issued on all three DGE queues (sync/scalar HWDGE, gpsimd
  SWDGE) in consumption order: the first x chunk and W1[m0] go first
  and land ~8.2us, weight tiles stay >=2 m-chunks ahead of the PE, x
  and W2 for later blocks stage during block-0 compute, outputs flush
  per 512-token block on the gpsimd queue during compute.
- The L1->L2 transition inside each token block stalls the PE on the
  last hidden chunk's silu*mul (vector op) before the first W2 matmul
  group can accumulate; the first W1 group of the NEXT block is
  emitted in between to cover that latency (and symmetrically removes
  the L2->L1 bubble). The last block has no successor, so its output
  matmul groups are split into 256-token halves: the first half's
  store chain starts ~1us before the final matmul retires, shrinking
  the post-compute flush tail (DMA-to-HBM completion receipt ~1us).

On-device compute uses bf16 matmuls (full-rate on the TRN2 PE, ~5e-3
relative error vs the 2e-2 gate) with fp32 PSUM accumulation; fp8
DoubleRow (1.44x) was measured numerically to land at 4.5-6.8e-2 error
in every variant, over the gate, so bf16 is the fastest legal dtype.
Activations live transposed ([feature, token]) on device so every
matmul consumes naturally-laid-out weights as the stationary operand
and no on-device transposes are needed. Weights are host-permuted
m-major so each DMA fetches exactly the 128-column block the next
psum group needs; x/y are host-permuted (k,block)/(block,j)-major so
every transfer is a contiguous 128KB DRAM range.
"""

import os
import sys
import types

for _p in ("/opt/trn_rl_repo",):
    if os.path.isdir(_p) and _p not in sys.path:
        sys.path.insert(0, _p)

import numpy as np
import ml_dtypes

BF16 = ml_dtypes.bfloat16

# Problem dims (fixed by the nn.Module spec)
D = 512          # d_model
H = 1024         # ffn hidden
E = 8            # experts
TOPK = 2
T = 8192         # tokens = 4*2048
P = 128          # SBUF partitions
CAP = 2048       # per-expert token capacity (capacity factor 1.0)
BT = 512         # token block (moving operand / PSUM bank limit)
NB = CAP // BT   # 4 token blocks of 512
DK = D // P      # 4 contraction chunks over d
MH = H // P      # 8 hidden chunks
N_CORES = 8

_compiled = {}
last_exec_time_ns = None
last_results = None


def _install_axon_trace_shim():
    """Make trace=True under axon survive images without antenv.axon_hooks."""
    try:
        import antenv  # noqa: F401
    except Exception:
        return
    try:
        from antenv import axon_hooks  # noqa: F401
        return  # real module present
    except Exception:
        pass
    try:
        import antenv
        boot_dir = "/root/.axon_site/trn_agent_boot"
        if os.path.isdir(boot_dir) and boot_dir not in sys.path:
            sys.path.insert(0, boot_dir)
        import trn_boot
        mod = types.ModuleType("antenv.axon_hooks")
        holder = {"hook": trn_boot._ntff_profile_via_ctypes("/opt/axon/libaxon_pjrt.so")}
        mod.set_axon_ntff_profile_hook = lambda h: holder.__setitem__("hook", h)
        mod.get_axon_ntff_profile_hook = lambda: holder["hook"]
        sys.modules["antenv.axon_hooks"] = mod
        antenv.axon_hooks = mod
    except Exception:
        pass


def _patch_upload_artifacts():
    """Artifact upload needs fishnet; degrade to the local dir if absent."""
    try:
        import concourse.bass_utils as bu
        orig = bu.upload_artifacts

        def safe_upload(tmpdir):
            try:
                return orig(tmpdir)
            except Exception:
                return tmpdir

        if getattr(bu.upload_artifacts, "__name__", "") != "safe_upload":
            bu.upload_artifacts = safe_upload
    except Exception:
        pass


def _build():
    from concourse import bacc, mybir
    import concourse.tile as tile

    f32 = mybir.dt.float32
    bf16 = mybir.dt.bfloat16

    nc = bacc.Bacc(num_swdge_queues=1)
    # x chunk-major: (k, b) block is a contiguous [P, BT] 128KB DRAM range
    xd = nc.declare_dram_parameter("xd", [DK * NB * P, BT], bf16, isOutput=False)
    w1 = nc.declare_dram_parameter("w1", [MH * P, DK * P], bf16, isOutput=False)
    w3 = nc.declare_dram_parameter("w3", [MH * P, DK * P], bf16, isOutput=False)
    w2 = nc.declare_dram_parameter("w2", [MH * P, D], bf16, isOutput=False)
    # y block-major: (b, j) block contiguous
    yd = nc.declare_dram_parameter("yd", [NB * DK * P, BT], bf16, isOutput=True)

    with tile.TileContext(nc) as tc:
        with tc.tile_pool(name="wpool", bufs=1) as wpool, \
             tc.tile_pool(name="act", bufs=2) as act, \
             tc.tile_pool(name="psum", bufs=1, space="PSUM") as psum:

            w1s = wpool.tile([P, MH, DK * P], bf16, tag="w1s")
            w3s = wpool.tile([P, MH, DK * P], bf16, tag="w3s")
            w2s = wpool.tile([P, MH, D], bf16, tag="w2s")
            xs = wpool.tile([P, DK, CAP], bf16, tag="xs")

            xv = xd[:].rearrange("(k b p) t -> k b p t", b=NB, p=P)
            w1v = w1[:].rearrange("(m p) c -> m p c", p=P)
            w3v = w3[:].rearrange("(m p) c -> m p c", p=P)
            w2v = w2[:].rearrange("(m p) d -> m p d", p=P)
            yv = yd[:].rearrange("(b j p) t -> b j p t", j=DK, p=P)

            def stage_x(eng, k, b):
                eng.dma_start(out=xs[:, k, BT * b:BT * (b + 1)], in_=xv[k, b])

            # PE warmup weights; memset on gpsimd (ready earliest, and
            # keeps vector/scalar free for their first real ops).
            wscr = wpool.tile([P, P], bf16, tag="wscr")
            nc.gpsimd.memset(wscr[:], 0)
            warm_ps = psum.tile([P, P], f32, tag="warm")

            def warm(n):
                for _ in range(n):
                    nc.tensor.matmul(out=warm_ps[:], lhsT=wscr[:], rhs=wscr[:],
                                     start=True, stop=True)

            # First DMA wave, in PE consumption order, spread over the
            # three DGE queues (engine FIFO order == emission order).
            # The scalar queue gets ONLY two early transfers: every
            # dma_start occupies its sequencer ~600ns, and the silu
            # chain must not sit behind a pile of issues (the PE can
            # run at most 2 hidden-chunks ahead of silu/mul on the
            # ps1/ps2 slots, so a lagging scalar stalls the PE).
            stage_x(nc.sync, 0, 0)
            nc.scalar.dma_start(out=w1s[:, 0], in_=w1v[0])
            stage_x(nc.gpsimd, 3, 0)
            # Dummy matmuls keep the PE busy (HAM activity window) while
            # the first operands are in flight (~8.2us arrival).
            warm(10)
            stage_x(nc.sync, 1, 0)
            stage_x(nc.scalar, 2, 0)
            nc.gpsimd.dma_start(out=w3s[:, 0], in_=w3v[0])
            nc.gpsimd.dma_start(out=w3s[:, 1], in_=w3v[1])

            ht = wpool.tile([P, MH, BT], bf16, tag="ht", bufs=2)

            def l1_group(tag, ws, m, b, fills=None):
                ps = psum.tile([P, BT], f32, tag=tag, bufs=2)
                tok = slice(BT * b, BT * (b + 1))
                for k in range(DK):
                    nc.tensor.matmul(out=ps[:], lhsT=ws[:, m, k * P:(k + 1) * P],
                                     rhs=xs[:, k, tok],
                                     start=(k == 0), stop=(k == DK - 1))
                    if fills:
                        warm(fills[k])
                return ps

            pending_ps1 = None
            for b in range(NB):
                for m in range(MH):
                    if pending_ps1 is not None:
                        ps1, pending_ps1 = pending_ps1, None
                    elif b == 0 and m == 0:
                        # warmup matmuls interleave into the arrival
                        # gaps of the bandwidth-paced first wave
                        ps1 = l1_group("ps1", w1s, m, b, fills=(3, 3, 2, 2))
                    else:
                        ps1 = l1_group("ps1", w1s, m, b)
                    if b == 0:
                        # Weight ladder ~2 chunks ahead of consumption,
                        # emitted AFTER the groups that must not pick up
                        # a (tile-granular) dependency on them.
                        if m + 1 < MH:
                            nc.sync.dma_start(out=w1s[:, m + 1], in_=w1v[m + 1])
                        if m + 2 < MH:
                            nc.gpsimd.dma_start(out=w3s[:, m + 2], in_=w3v[m + 2])
                        if 1 <= m <= 3:
                            stage_x(nc.gpsimd, m, 1)
                        if m == 2:
                            stage_x(nc.sync, 0, 1)
                        if 3 <= m <= 6:
                            nc.sync.dma_start(out=w2s[:, 2 * (m - 3)],
                                              in_=w2v[2 * (m - 3)])
                        if 4 <= m <= 7:
                            nc.gpsimd.dma_start(out=w2s[:, 2 * (m - 4) + 1],
                                                in_=w2v[2 * (m - 4) + 1])
                    elif b + 1 < NB and 1 <= m <= 4:
                        # stage next block's x during this block's L1
                        stage_x(nc.gpsimd, m - 1, b + 1)
                    sil = act.tile([P, BT], f32, tag="sil")
                    nc.scalar.activation(sil[:], ps1[:],
                                         mybir.ActivationFunctionType.Silu)
                    ps2 = l1_group("ps2", w3s, m, b)
                    if b == NB - 1 and m == MH - 1:
                        # split the last mul so the first half-width L2
                        # group can start ~350ns earlier
                        for h in range(2):
                            hs = slice(256 * h, 256 * (h + 1))
                            nc.vector.tensor_mul(out=ht[:, m, hs], in0=sil[:, hs],
                                                 in1=ps2[:, hs])
                    else:
                        nc.vector.tensor_mul(out=ht[:, m], in0=sil[:], in1=ps2[:])

                if b + 1 < NB:
                    # Borrow the next block's first W1 group: covers the
                    # silu*mul latency of this block's last hidden chunk
                    # so the first psy matmul never stalls the PE.
                    pending_ps1 = l1_group("ps1", w1s, 0, b + 1)

                tok = slice(BT * b, BT * (b + 1))
                if b < NB - 1:
                    for j in range(DK):
                        psy = psum.tile([P, BT], f32, tag="psy", bufs=2)
                        for m in range(MH):
                            nc.tensor.matmul(out=psy[:], lhsT=w2s[:, m, j * P:(j + 1) * P],
                                             rhs=ht[:, m, :],
                                             start=(m == 0), stop=(m == MH - 1))
                        yt = act.tile([P, BT], bf16, tag="yt")
                        nc.vector.tensor_copy(out=yt[:], in_=psy[:])
                        nc.gpsimd.dma_start(out=yv[b, j], in_=yt[:])
                else:
                    # Last block: half-width (256-token) L2 groups so the
                    # store chain overlaps the final matmuls and the very
                    # last transfer is small + issued on an idle HW queue.
                    for j in range(DK):
                        for h in range(2):
                            hs = slice(256 * h, 256 * (h + 1))
                            psy = psum.tile([P, 256], f32, tag="psy", bufs=2)
                            for m in range(MH):
                                nc.tensor.matmul(out=psy[:],
                                                 lhsT=w2s[:, m, j * P:(j + 1) * P],
                                                 rhs=ht[:, m, hs],
                                                 start=(m == 0), stop=(m == MH - 1))
                            yt = act.tile([P, 256], bf16, tag="yth", bufs=8)
                            nc.vector.tensor_copy(out=yt[:], in_=psy[:])
                            # First two j's ride gpsimd (their receipts
                            # finish during j2/j3 compute, off the
                            # barrier's critical path); the HWDGE rings
                            # stay 2-deep so the final j3 pair's
                            # completion receipt gates the drain ASAP.
                            if j < 2:
                                nc.gpsimd.dma_start(out=yv[b, j][:, hs], in_=yt[:])
                            elif j == 2:
                                eng = nc.sync if h == 0 else nc.scalar
                                eng.dma_start(out=yv[b, j][:, hs], in_=yt[:])
                            else:
                                eng = nc.scalar if h == 0 else nc.sync
                                eng.dma_start(out=yv[b, j][:, hs], in_=yt[:])

    nc.compile()
    return nc


def _route(x2d, Wg, bg):
    """Replicate the reference router on host.

    Selection runs in float64 (agrees with the reference's fp32 jax
    selection whenever top-2/top-3 logit gaps exceed fp32 matmul noise,
    which holds with >10x margin on this distribution); the softmax over
    the two selected logits runs in fp32 like the reference.
    """
    logits64 = x2d.astype(np.float64) @ Wg.astype(np.float64) + bg.astype(np.float64)
    i1 = np.argmax(logits64, axis=1)
    r = np.arange(T)
    masked = logits64.copy()
    masked[r, i1] = -np.inf
    i2 = np.argmax(masked, axis=1)

    # fp32 logit values for the softmax (match reference arithmetic)
    logits32 = (x2d @ Wg + bg).astype(np.float32)
    v1 = logits32[r, i1]
    v2 = logits32[r, i2]
    # softmax over [v1, v2] with v1 >= v2 (fp32)
    e2 = np.exp((v2 - v1).astype(np.float32))
    p1 = (1.0 / (1.0 + e2)).astype(np.float32)
    p2 = (e2 / (1.0 + e2)).astype(np.float32)
    return i1, i2, p1, p2


def _ffn_host(x2d, idx, W1e, W3e, W2e):
    """Exact fp32 SwiGLU FFN for a small set of tokens (overflow path)."""
    z = x2d[idx] @ W1e
    h = (z / (1.0 + np.exp(-z))) * (x2d[idx] @ W3e)
    return h @ W2e


def kernel(x, Wg, bg, W1, W3, W2):
    global last_exec_time_ns
    _install_axon_trace_shim()
    _patch_upload_artifacts()
    from concourse.bass_utils import run_bass_kernel_spmd

    x = np.asarray(x, np.float32)
    Wg = np.asarray(Wg, np.float32)
    bg = np.asarray(bg, np.float32)
    W1 = np.asarray(W1, np.float32)
    W3 = np.asarray(W3, np.float32)
    W2 = np.asarray(W2, np.float32)

    B, S, _ = x.shape
    x2d = np.ascontiguousarray(x.reshape(T, D))

    i1, i2, p1, p2 = _route(x2d, Wg, bg)

    # Dispatch: build each expert's token list + gate weights. Tokens past
    # CAP (load-imbalance remainder) fall to the exact host path.
    idx_lists, gate_lists = [], []
    spill_lists = []
    for e in range(E):
        m1 = i1 == e
        m2 = i2 == e
        idx = np.concatenate([np.nonzero(m1)[0], np.nonzero(m2)[0]])
        g = np.concatenate([p1[m1], p2[m2]]).astype(np.float32)
        if len(idx) > CAP:
            # Spill the smallest-gate tokens: they matter least if anything
            # about the two paths' rounding ever differs.
            order = np.argsort(-g, kind="stable")
            idx, g = idx[order], g[order]
            spill_lists.append((idx[CAP:], g[CAP:]))
            idx, g = idx[:CAP], g[:CAP]
        else:
            spill_lists.append((idx[:0], g[:0]))
        idx_lists.append(idx)
        gate_lists.append(g)

    x2dT_bf = np.ascontiguousarray(x2d.T.astype(BF16))  # [D, T]
    in_maps = []
    for e in range(E):
        idx = idx_lists[e]
        xe = np.zeros((D, CAP), BF16)
        xe[:, : len(idx)] = x2dT_bf[:, idx]
        # chunk-major x: [DK, NB, P, BT] so each (k, b) DMA is contiguous
        xkb = np.ascontiguousarray(
            xe.reshape(DK, P, NB, BT).transpose(0, 2, 1, 3)
        ).reshape(DK * NB * P, BT)
        # m-major weight layout: [MH, P, DK*128]
        w1m = np.ascontiguousarray(
            W1[e].astype(BF16).reshape(DK, P, MH, P).transpose(2, 1, 0, 3)
        ).reshape(MH * P, DK * P)
        w3m = np.ascontiguousarray(
            W3[e].astype(BF16).reshape(DK, P, MH, P).transpose(2, 1, 0, 3)
        ).reshape(MH * P, DK * P)
        in_maps.append({
            "xd": xkb,
            "w1": w1m,
            "w3": w3m,
            "w2": np.ascontiguousarray(W2[e].astype(BF16)),
        })

    if "nc" not in _compiled:
        _compiled["nc"] = _build()
    nc = _compiled["nc"]

    trace = bool(os.environ.get("BASS_TRACE"))
    res = run_bass_kernel_spmd(nc, in_maps, list(range(N_CORES)), trace=trace)
    last_exec_time_ns = res.exec_time_ns
    globals()["last_results"] = res

    y = np.zeros((T, D), np.float32)
    for e in range(E):
        idx = idx_lists[e]
        n = len(idx)
        # yd is [NB, DK, P, BT] block-major; back to [D, CAP]
        ye = np.asarray(res.results[e]["yd"]).reshape(NB, DK, P, BT)
        ye = ye.transpose(1, 2, 0, 3).reshape(D, CAP)
        y[idx] += gate_lists[e][:, None] * ye[:, :n].T.astype(np.float32)
        sidx, sg = spill_lists[e]
        if len(sidx):
            y[sidx] += sg[:, None] * _ffn_host(x2d, sidx, W1[e], W3[e], W2[e])
    return y.reshape(B, S, D)
